# revision 26
# baseline (speedup 1.0000x reference)
"""Trainium2 Bass kernel for nn_EdgeFeatCat (gnn_message_passing).

Math (see docstring of reference):
    h   = relu(node_feats @ Wn.T + bn)                  [B, N, E]
    fusion[e] = concat(h[bi, ii], h[bi, jj])            [Edges, 2E]
    joint = relu(fusion @ Wj.T + bj)                    [Edges, E]
    q   = relu(cond_feats @ Wq.T + bq)                  [B, E]
    out = relu(concat(joint, q[pos_b]) @ Wl.T + bl)     [Edges, E]

Key algebraic restructuring (exact, fp32):
    joint = relu(G1[row1] + G2[row2] + bj),  G1 = h@Wj1.T, G2 = h@Wj2.T
    out   = relu(joint @ Wl1.T + r[pos_b]),  r = q@Wl2.T + bl
With the canonical edge list (all ordered pairs i!=j, batch-major), the
gather is a broadcast: per batch, T[i*N+j] = relu(G1[i] + G2[j] + bj).
The kernel computes the full NxN pair tensor per batch on device (in a
feature-major "transposed" layout so the 128-wide feature dim sits on
SBUF partitions), applies the second linear + bias + relu, transposes
back to row-major via the tensor engine, and DMAs contiguous rows out.
The host drops the N diagonal rows per batch while unsharding.

Sharding: data-parallel over batch; 4 batches per core on 8 cores.
"""

import numpy as np

import concourse.bass as bass
import concourse.bacc as bacc
import concourse.mybir as mybir
import concourse.tile as tile
from concourse.bass_utils import run_bass_kernel_spmd

# ---- problem constants (hardcoded; kernel.py must be self-contained) ----
B, N, NODE_DIM, COND_DIM, E = 32, 96, 2048, 1024, 128
N_CORES = 8
B_LOC = B // N_CORES            # batches per core = 4
NN = N * N                      # 9216 pair slots per batch (incl. diagonal)
NODES_LOC = B_LOC * N           # 384 nodes per core
K_CHUNKS = NODE_DIM // 128      # 16
ROWS_LOC = B_LOC * NN           # 36864 device output rows per core
EDGES_PER_B = N * (N - 1)       # 9120

F32 = mybir.dt.float32
F32R = mybir.dt.float32r
AF = mybir.ActivationFunctionType
ALU = mybir.AluOpType

# engine-split tunables
CONSTRUCT_ACT_MOD = 0      # i % 3 == this -> ACT; set <0 to put all on DVE
EVICT2_DVE_MOD = 2         # chunk % EVICT2_DVE_MOD == 0 -> DVE else ACT

TRACE = False              # set True (e.g. from test.py) to profile
LAST_RESULT = None         # BassKernelResults of the last run
LAST_EXEC_NS = None


def _install_profile_hook():
    """Provide antenv.axon_hooks (absent in this image) so bass_utils can
    NTFF-profile under axon. Safe no-op on failure."""
    import sys
    import types
    try:
        from antenv.axon_hooks import get_axon_ntff_profile_hook  # noqa: F401
        return True
    except ImportError:
        pass
    try:
        import antenv
        from trn_agent_boot.trn_boot import _ntff_profile_via_ctypes
        hook = _ntff_profile_via_ctypes("/opt/axon/libaxon_pjrt.so")
        mod = types.ModuleType("antenv.axon_hooks")
        _state = {"hook": hook}
        mod.set_axon_ntff_profile_hook = lambda h: _state.__setitem__("hook", h)
        mod.get_axon_ntff_profile_hook = lambda: _state["hook"]
        sys.modules["antenv.axon_hooks"] = mod
        antenv.axon_hooks = mod
        return True
    except Exception as e:  # pragma: no cover
        print(f"kernel: profiling hook unavailable ({e}); running untraced")
        return False


import os as _os
# float32r: 4x faster PE streaming but ~1.6e-4 absmax-rel error.
# KERNEL_F32R: 0=none (exact fp32), 1=everywhere, 2=final matmul only,
# 3=prologue (node/h/G) only
_F32R_MODE = int(_os.environ.get("KERNEL_F32R", "0"))
USE_F32R = _F32R_MODE != 0

# PRODT: node/wnT operands of the h matmul (always DMA-fed, safe as f32r).
# h_sb stays fp32 (engine-written float32r hard-crashed the exec unit), so
# the small G matmuls stay fp32 too.
PRODT = F32R if _F32R_MODE in (1, 3) else F32
FINDT = F32R if _F32R_MODE in (1, 2) else F32  # tT / wl1T final chain

_ADD_RELU_OP = None


def _get_add_relu_op():
    """Register a fused out = relu(in0 + in1) custom DVE op at runtime.

    Saves a whole second pass over the [128, NN] pair tensor (the relu)
    per batch. Registration mutates concourse.dve_ops module state for
    this process only; the uops sha is pinned from the lowering itself.
    """
    global _ADD_RELU_OP
    if _ADD_RELU_OP is not None:
        return _ADD_RELU_OP
    import re
    import concourse.dve_ops as dve_ops
    from concourse.dve_ops import DveOp
    from concourse.dve_spec import Spec, Src0, Src1, relu
    from concourse.dve_table_gen import dve_ver_for

    name = "TT_ADD_RELU_EFC"
    existing = {op.name: op for op in dve_ops.OPS}
    if name in existing:
        _ADD_RELU_OP = existing[name]
        return _ADD_RELU_OP
    op = DveOp(
        name,
        Spec(
            body=relu(Src0 + Src1),
            reference=lambda in0, in1, s0, s1, imm2: np.maximum(
                in0.astype(np.float32) + in1, 0.0).astype(np.float32),
        ),
        subdim=False,
        uops_sha={},
    )
    dve_ops.OPS.append(op)
    dve_ops.CUSTOM_DVE_SPECS[name] = op.spec
    dve_ops._SUB_OPCODE_FOR_NAME[name] = (
        max(dve_ops._SUB_OPCODE_FOR_NAME.values()) + 1)
    assert dve_ops._SUB_OPCODE_FOR_NAME[name] < 0x20
    ver = dve_ver_for("TRN2")
    try:
        op.compile(ver)
    except ValueError as e:
        got = re.search(rf"\({ver}: (\w+) ", str(e)).group(1)
        op.uops_sha[ver] = got
        op.compile(ver)
    _ADD_RELU_OP = op
    return op


def _emit(tc, nc, aps):
    node, wnT, wj1T, wj2T, wl1T, bn, bj, rT, out = aps
    from contextlib import ExitStack

    with ExitStack() as ctx:
        # ---------------- constant / persistent tiles ----------------
        consts = ctx.enter_context(tc.tile_pool(name="consts", bufs=1))
        wnT_sb = consts.tile([128, NODE_DIM], PRODT, tag="wnT")
        for k in range(K_CHUNKS):
            nc.sync.dma_start(wnT_sb[:, k * 128:(k + 1) * 128],
                              wnT[k * 128:(k + 1) * 128, :])
        wj1T_sb = consts.tile([128, E], F32, tag="wj1T")
        nc.sync.dma_start(wj1T_sb[:], wj1T[:])
        wj2T_sb = consts.tile([128, E], F32, tag="wj2T")
        nc.sync.dma_start(wj2T_sb[:], wj2T[:])
        wl1T_sb = consts.tile([128, E], FINDT, tag="wl1T")
        nc.sync.dma_start(wl1T_sb[:], wl1T[:])
        bn_sb = consts.tile([128, 1], F32, tag="bn")
        nc.sync.dma_start(bn_sb[:], bn[:])
        bj_sb = consts.tile([128, 1], F32, tag="bj")
        nc.sync.dma_start(bj_sb[:], bj[:])
        rT_sb = consts.tile([128, B_LOC], F32, tag="rT")
        nc.sync.dma_start(rT_sb[:], rT[:])

        h_sb = consts.tile([128, NODES_LOC], F32, tag="h")
        g1_sb = consts.tile([128, NODES_LOC], F32, tag="g1")
        g2_sb = consts.tile([128, NODES_LOC], F32, tag="g2")

        # ---------------- prologue: h, G1, G2 ----------------
        with ExitStack() as pctx:
            ntp = pctx.enter_context(tc.tile_pool(name="nodeT", bufs=1))
            pmm = pctx.enter_context(
                tc.tile_pool(name="pmm", bufs=2, space="PSUM"))

            # node arrives pre-transposed (feature-major) from the host
            nodeT_sb = ntp.tile([128, K_CHUNKS * NODES_LOC], PRODT, tag="nodeT")
            for k in range(K_CHUNKS):
                nc.sync.dma_start(
                    nodeT_sb[:, k * NODES_LOC:(k + 1) * NODES_LOC],
                    node[k * 128:(k + 1) * 128, :])

            # h^T = relu(Wn @ node^T + bn)   [128(E), 384]
            h_ps = pmm.tile([128, NODES_LOC], F32, tag="hps")
            for k in range(K_CHUNKS):
                nc.tensor.matmul(
                    h_ps[:],
                    (wnT_sb[:, k * 128:(k + 1) * 128]),
                    (nodeT_sb[:, k * NODES_LOC:(k + 1) * NODES_LOC]),
                    start=(k == 0), stop=(k == K_CHUNKS - 1),
                )
            nc.scalar.activation(h_sb[:], h_ps[:], AF.Relu,
                                 bias=bn_sb[:, 0:1], scale=1.0)

            # G1^T = Wj1 @ h^T + bj ; G2^T = Wj2 @ h^T
            g1_ps = pmm.tile([128, NODES_LOC], F32, tag="gps")
            nc.tensor.matmul(g1_ps[:], (wj1T_sb[:]), (h_sb[:]),
                             start=True, stop=True)
            nc.scalar.activation(g1_sb[:], g1_ps[:], AF.Identity,
                                 bias=bj_sb[:, 0:1], scale=1.0)
            g2_ps = pmm.tile([128, NODES_LOC], F32, tag="gps")
            nc.tensor.matmul(g2_ps[:], (wj2T_sb[:]), (h_sb[:]),
                             start=True, stop=True)
            nc.scalar.copy(g2_sb[:], g2_ps[:])

        # ---------------- main: per-batch pair tensor + final linear ----
        # T^T[:, i*N+j] = relu(G1[:,i] + G2[:,j]); O^T = relu(Wl1@T^T + r_b)
        # is written per batch as [128, NN] feature-major; the host
        # transposes + drops diagonals while unsharding (a device-side
        # transpose would burn PE cycles and shatter the output DMA into
        # 512-byte descriptors).
        tpool = ctx.enter_context(tc.tile_pool(name="tT", bufs=2))
        oT_pool = ctx.enter_context(tc.tile_pool(name="oT", bufs=2))
        poT = ctx.enter_context(tc.tile_pool(name="poT", bufs=3, space="PSUM"))

        n_chunks = NN // 1024   # 9 double-bank chunks of 1024 pair-columns
        n_pieces = 3            # construct granularity: 32 i's = 3072 cols
        ip = N // n_pieces      # i's per piece
        pw = ip * N             # cols per piece
        for b in range(B_LOC):
            tT = tpool.tile([128, NN], FINDT, tag="tT")
            g1b = g1_sb[:, b * N:(b + 1) * N]
            g2b = g2_sb[:, b * N:(b + 1) * N]
            # T = relu(G1 (bcast over j) + G2 (bcast over i)) in a single
            # fused custom-DVE pass per piece
            g2rep = g2b.unsqueeze(1).broadcast_to([128, ip, N])
            for p in range(n_pieces):
                piece = tT[:, p * pw:(p + 1) * pw]
                out3d = piece.rearrange("q (i j) -> q i j", j=N)
                g1rep = (g1b[:, p * ip:(p + 1) * ip]
                         .unsqueeze(2).broadcast_to([128, ip, N]))
                nc.vector._custom_dve(_get_add_relu_op(), out=out3d,
                                      in0=g1rep, in1=g2rep)

            oT = oT_pool.tile([128, NN], F32, tag="oT")
            for c in range(n_chunks):
                oT_ps = poT.tile([128, 1024], F32, tag="oTps")
                for hh in range(2):
                    nc.tensor.matmul(
                        oT_ps[:, hh * 512:(hh + 1) * 512], (wl1T_sb[:]),
                        (tT[:, c * 1024 + hh * 512: c * 1024 + (hh + 1) * 512]),
                        start=True, stop=True)
                # relu + per-batch bias r while evicting PSUM->SBUF
                dst = oT[:, c * 1024:(c + 1) * 1024]
                nc.scalar.activation(dst, oT_ps[:], AF.Relu,
                                     bias=rT_sb[:, b:b + 1], scale=1.0)
            third = NN // 3
            for rpt in range(3):
                nc.sync.dma_start(
                    out[b][:, rpt * third:(rpt + 1) * third],
                    oT[:, rpt * third:(rpt + 1) * third])


_PROGRAM = None


def _build_program():
    global _PROGRAM
    if _PROGRAM is not None:
        return _PROGRAM
    nc = bacc.Bacc("TRN2", target_bir_lowering=False, debug=False)
    aps = (
        nc.dram_tensor("node", [NODE_DIM, NODES_LOC], PRODT, kind="ExternalInput").ap(),
        nc.dram_tensor("wnT", [NODE_DIM, E], PRODT, kind="ExternalInput").ap(),
        nc.dram_tensor("wj1T", [E, E], F32, kind="ExternalInput").ap(),
        nc.dram_tensor("wj2T", [E, E], F32, kind="ExternalInput").ap(),
        nc.dram_tensor("wl1T", [E, E], FINDT, kind="ExternalInput").ap(),
        nc.dram_tensor("bn", [E, 1], F32, kind="ExternalInput").ap(),
        nc.dram_tensor("bj", [E, 1], F32, kind="ExternalInput").ap(),
        nc.dram_tensor("rT", [E, B_LOC], F32, kind="ExternalInput").ap(),
        nc.dram_tensor("outT", [B_LOC, E, NN], F32, kind="ExternalOutput").ap(),
    )
    with tile.TileContext(nc) as tc:
        _emit(tc, nc, aps)
    nc.compile()
    _PROGRAM = nc
    return nc


def _norm_weight(v, g):
    return (v * (g / np.linalg.norm(v, axis=1))[:, None]).astype(np.float32)


def _canonical_edges():
    i, j = np.meshgrid(np.arange(N), np.arange(N), indexing="ij")
    pair = (i * N + j)[i != j]                       # [N*(N-1)]
    edges = (np.arange(B)[:, None] * NN + pair[None, :]).reshape(-1)
    return edges.astype(np.int32), pair


def _reference_numpy(node_feats, cond_feats, edge_indexes, Wn, bn, Wj, bj,
                     Wq, bq, Wl, bl):
    """general fallback for non-canonical edge index tensors (host compute)"""
    h = np.maximum(node_feats.reshape(-1, NODE_DIM) @ Wn.T + bn, 0.0)
    e = edge_indexes.astype(np.int64)
    bi = e // NN
    rem = e % NN
    ii = rem // N
    jj = rem % N
    fusion = np.concatenate([h[bi * N + ii], h[bi * N + jj]], axis=1)
    joint = np.maximum(fusion @ Wj.T + bj, 0.0)
    q = np.maximum(cond_feats @ Wq.T + bq, 0.0)
    e_per = joint.shape[0] // B
    r = q @ Wl[:, E:].T + bl
    out = np.maximum(
        joint @ Wl[:, :E].T + np.repeat(r, e_per, axis=0), 0.0)
    return out.astype(np.float32)


def kernel(node_feats, cond_feats, edge_indexes,
           node_v, node_g, node_b,
           joint_v, joint_g, joint_b,
           q_v, q_g, q_b,
           lin_v, lin_g, lin_b):
    global LAST_RESULT, LAST_EXEC_NS
    node_feats = np.asarray(node_feats, dtype=np.float32)
    cond_feats = np.asarray(cond_feats, dtype=np.float32)
    edge_indexes = np.asarray(edge_indexes)

    Wn = _norm_weight(np.asarray(node_v, np.float32), np.asarray(node_g, np.float32))
    Wj = _norm_weight(np.asarray(joint_v, np.float32), np.asarray(joint_g, np.float32))
    Wq = _norm_weight(np.asarray(q_v, np.float32), np.asarray(q_g, np.float32))
    Wl = _norm_weight(np.asarray(lin_v, np.float32), np.asarray(lin_g, np.float32))
    bn = np.asarray(node_b, np.float32)
    bj = np.asarray(joint_b, np.float32)
    bq = np.asarray(q_b, np.float32)
    bl = np.asarray(lin_b, np.float32)

    canon, pair = _canonical_edges()
    if edge_indexes.shape != canon.shape or not np.array_equal(
            edge_indexes.astype(np.int64), canon.astype(np.int64)):
        print("kernel: non-canonical edge_indexes -> host fallback path")
        return _reference_numpy(node_feats, cond_feats, edge_indexes,
                                Wn, bn, Wj, bj, Wq, bq, Wl, bl)

    # host-side tiny precompute: per-batch bias r = q @ Wl2.T + bl  [B, E]
    q = np.maximum(cond_feats @ Wq.T + bq, 0.0)
    r = (q @ Wl[:, E:].T + bl).astype(np.float32)

    wnT = np.ascontiguousarray(Wn.T)                  # [2048, 128]
    wj1T = np.ascontiguousarray(Wj[:, :E].T)          # [128, 128]
    wj2T = np.ascontiguousarray(Wj[:, E:].T)
    wl1T = np.ascontiguousarray(Wl[:, :E].T)
    bn2 = bn.reshape(E, 1).copy()
    bj2 = bj.reshape(E, 1).copy()

    nc = _build_program()
    in_maps = []
    for c in range(N_CORES):
        # feature-major (transposed) node layout for this core's batches
        nl = np.ascontiguousarray(
            node_feats[c * B_LOC:(c + 1) * B_LOC].reshape(NODES_LOC, NODE_DIM).T)
        rT = np.ascontiguousarray(r[c * B_LOC:(c + 1) * B_LOC].T)  # [128, 4]
        in_maps.append({
            "node": nl, "wnT": wnT, "wj1T": wj1T, "wj2T": wj2T,
            "wl1T": wl1T, "bn": bn2, "bj": bj2, "rT": rT,
        })

    do_trace = TRACE and _install_profile_hook()
    res = run_bass_kernel_spmd(nc, in_maps, core_ids=list(range(N_CORES)),
                               trace=do_trace)
    LAST_RESULT = res
    LAST_EXEC_NS = res.exec_time_ns

    # unshard: cores x [4, 128, 9216] feature-major -> row-major, drop
    # the diagonal pair slots, flatten
    full = np.concatenate([m["outT"] for m in res.results], axis=0)
    out = full.transpose(0, 2, 1)[:, pair, :].reshape(B * EDGES_PER_B, E)
    return np.ascontiguousarray(out)


# revision 29
# speedup vs baseline: 1.1179x; 1.1179x over previous
"""Trainium2 Bass kernel for nn_EdgeFeatCat (gnn_message_passing).

Math (see docstring of reference):
    h   = relu(node_feats @ Wn.T + bn)                  [B, N, E]
    fusion[e] = concat(h[bi, ii], h[bi, jj])            [Edges, 2E]
    joint = relu(fusion @ Wj.T + bj)                    [Edges, E]
    q   = relu(cond_feats @ Wq.T + bq)                  [B, E]
    out = relu(concat(joint, q[pos_b]) @ Wl.T + bl)     [Edges, E]

Key algebraic restructuring (exact, fp32):
    joint = relu(G1[row1] + G2[row2] + bj),  G1 = h@Wj1.T, G2 = h@Wj2.T
    out   = relu(joint @ Wl1.T + r[pos_b]),  r = q@Wl2.T + bl
With the canonical edge list (all ordered pairs i!=j, batch-major), the
gather is a broadcast: per batch, T[i*N+j] = relu(G1[i] + G2[j] + bj).
The kernel computes the full NxN pair tensor per batch on device (in a
feature-major "transposed" layout so the 128-wide feature dim sits on
SBUF partitions), applies the second linear + bias + relu, transposes
back to row-major via the tensor engine, and DMAs contiguous rows out.
The host drops the N diagonal rows per batch while unsharding.

Sharding: data-parallel over batch; 4 batches per core on 8 cores.
"""

import numpy as np

import concourse.bass as bass
import concourse.bacc as bacc
import concourse.mybir as mybir
import concourse.tile as tile
from concourse.bass_utils import run_bass_kernel_spmd

# ---- problem constants (hardcoded; kernel.py must be self-contained) ----
B, N, NODE_DIM, COND_DIM, E = 32, 96, 2048, 1024, 128
N_CORES = 8
B_LOC = B // N_CORES            # batches per core = 4
NN = N * N                      # 9216 pair slots per batch (incl. diagonal)
NODES_LOC = B_LOC * N           # 384 nodes per core
K_CHUNKS = NODE_DIM // 128      # 16
ROWS_LOC = B_LOC * NN           # 36864 device output rows per core
EDGES_PER_B = N * (N - 1)       # 9120

F32 = mybir.dt.float32
F32R = mybir.dt.float32r
AF = mybir.ActivationFunctionType
ALU = mybir.AluOpType

# engine-split tunables
CONSTRUCT_ACT_MOD = 0      # i % 3 == this -> ACT; set <0 to put all on DVE
EVICT2_DVE_MOD = 2         # chunk % EVICT2_DVE_MOD == 0 -> DVE else ACT

TRACE = False              # set True (e.g. from test.py) to profile
LAST_RESULT = None         # BassKernelResults of the last run
LAST_EXEC_NS = None


def _install_profile_hook():
    """Provide antenv.axon_hooks (absent in this image) so bass_utils can
    NTFF-profile under axon. Safe no-op on failure."""
    import sys
    import types
    try:
        from antenv.axon_hooks import get_axon_ntff_profile_hook  # noqa: F401
        return True
    except ImportError:
        pass
    try:
        import antenv
        from trn_agent_boot.trn_boot import _ntff_profile_via_ctypes
        hook = _ntff_profile_via_ctypes("/opt/axon/libaxon_pjrt.so")
        mod = types.ModuleType("antenv.axon_hooks")
        _state = {"hook": hook}
        mod.set_axon_ntff_profile_hook = lambda h: _state.__setitem__("hook", h)
        mod.get_axon_ntff_profile_hook = lambda: _state["hook"]
        sys.modules["antenv.axon_hooks"] = mod
        antenv.axon_hooks = mod
        return True
    except Exception as e:  # pragma: no cover
        print(f"kernel: profiling hook unavailable ({e}); running untraced")
        return False


import os as _os
# float32r: 4x faster PE streaming but ~1.6e-4 absmax-rel error.
# KERNEL_F32R: 0=none (exact fp32), 1=everywhere, 2=final matmul only,
# 3=prologue (node/h/G) only
_F32R_MODE = int(_os.environ.get("KERNEL_F32R", "0"))
USE_F32R = _F32R_MODE != 0

# PRODT: node/wnT operands of the h matmul (always DMA-fed, safe as f32r).
# h_sb stays fp32 (engine-written float32r hard-crashed the exec unit), so
# the small G matmuls stay fp32 too.
PRODT = F32R if _F32R_MODE in (1, 3) else F32
FINDT = F32R if _F32R_MODE in (1, 2) else F32  # tT / wl1T final chain

_ADD_RELU_OP = None


def _get_add_relu_op():
    """Register a fused out = relu(in0 + in1) custom DVE op at runtime.

    Saves a whole second pass over the [128, NN] pair tensor (the relu)
    per batch. Registration mutates concourse.dve_ops module state for
    this process only; the uops sha is pinned from the lowering itself.
    """
    global _ADD_RELU_OP
    if _ADD_RELU_OP is not None:
        return _ADD_RELU_OP
    import re
    import concourse.dve_ops as dve_ops
    from concourse.dve_ops import DveOp
    from concourse.dve_spec import Spec, Src0, Src1, relu
    from concourse.dve_table_gen import dve_ver_for

    name = "TT_ADD_RELU_EFC"
    existing = {op.name: op for op in dve_ops.OPS}
    if name in existing:
        _ADD_RELU_OP = existing[name]
        return _ADD_RELU_OP
    op = DveOp(
        name,
        Spec(
            body=relu(Src0 + Src1),
            reference=lambda in0, in1, s0, s1, imm2: np.maximum(
                in0.astype(np.float32) + in1, 0.0).astype(np.float32),
        ),
        subdim=False,
        uops_sha={},
    )
    dve_ops.OPS.append(op)
    dve_ops.CUSTOM_DVE_SPECS[name] = op.spec
    dve_ops._SUB_OPCODE_FOR_NAME[name] = (
        max(dve_ops._SUB_OPCODE_FOR_NAME.values()) + 1)
    assert dve_ops._SUB_OPCODE_FOR_NAME[name] < 0x20
    ver = dve_ver_for("TRN2")
    try:
        op.compile(ver)
    except ValueError as e:
        got = re.search(rf"\({ver}: (\w+) ", str(e)).group(1)
        op.uops_sha[ver] = got
        op.compile(ver)
    _ADD_RELU_OP = op
    return op


def _emit(tc, nc, aps):
    node, wjT, wl1T, bias, out = aps
    from contextlib import ExitStack

    CW = 128 + NODES_LOC          # 512: [wnT_k | nodeT_k] packed width
    with ExitStack() as ctx:
        # ---------------- constant / persistent tiles ----------------
        consts = ctx.enter_context(tc.tile_pool(name="consts", bufs=1))
        wjT_sb = consts.tile([128, 2 * E], F32, tag="wjT")
        nc.sync.dma_start(wjT_sb[:], wjT[:])
        wj1T_sb = wjT_sb[:, 0:E]
        wj2T_sb = wjT_sb[:, E:2 * E]
        wl1T_sb = consts.tile([128, E], FINDT, tag="wl1T")
        nc.sync.dma_start(wl1T_sb[:], wl1T[:])
        bias_sb = consts.tile([128, 2 + B_LOC], F32, tag="bias")
        nc.sync.dma_start(bias_sb[:], bias[:])
        bn_sb = bias_sb[:, 0:1]
        bj_sb = bias_sb[:, 1:2]
        rT_sb = bias_sb[:, 2:2 + B_LOC]

        h_sb = consts.tile([128, NODES_LOC], F32, tag="h")
        g1_sb = consts.tile([128, NODES_LOC], F32, tag="g1")
        g2_sb = consts.tile([128, NODES_LOC], F32, tag="g2")

        # ---------------- prologue: h, G1, G2 ----------------
        with ExitStack() as pctx:
            ntp = pctx.enter_context(tc.tile_pool(name="nodeT", bufs=1))
            pmm = pctx.enter_context(
                tc.tile_pool(name="pmm", bufs=2, space="PSUM"))

            # node arrives host-packed per k-chunk: [wnT_k | nodeT_k]
            # (feature-major) so each h matmul chases one quarter-DMA
            comb_sb = ntp.tile([128, K_CHUNKS * CW], PRODT, tag="comb")
            KG = 4                     # k-chunks per DMA
            for g in range(K_CHUNKS // KG):
                src = node[g * KG * 128:(g + 1) * KG * 128, :]
                src3 = src.rearrange("(k p) c -> p k c", k=KG)
                dst = comb_sb[:, g * KG * CW:(g + 1) * KG * CW]
                dst3 = dst.rearrange("p (k c) -> p k c", k=KG)
                nc.sync.dma_start(dst3, src3)

            # h^T = relu(Wn @ node^T + bn)   [128(E), 384]
            h_ps = pmm.tile([128, NODES_LOC], F32, tag="hps")
            for k in range(K_CHUNKS):
                base = k * CW
                nc.tensor.matmul(
                    h_ps[:],
                    (comb_sb[:, base:base + 128]),
                    (comb_sb[:, base + 128:base + CW]),
                    start=(k == 0), stop=(k == K_CHUNKS - 1),
                )
            nc.scalar.activation(h_sb[:], h_ps[:], AF.Relu,
                                 bias=bn_sb, scale=1.0)

            # G1^T = Wj1 @ h^T + bj ; G2^T = Wj2 @ h^T
            g1_ps = pmm.tile([128, NODES_LOC], F32, tag="gps")
            nc.tensor.matmul(g1_ps[:], wj1T_sb, (h_sb[:]),
                             start=True, stop=True)
            nc.scalar.activation(g1_sb[:], g1_ps[:], AF.Identity,
                                 bias=bj_sb, scale=1.0)
            g2_ps = pmm.tile([128, NODES_LOC], F32, tag="gps")
            nc.tensor.matmul(g2_ps[:], wj2T_sb, (h_sb[:]),
                             start=True, stop=True)
            nc.scalar.copy(g2_sb[:], g2_ps[:])

        # ---------------- main: per-batch pair tensor + final linear ----
        # T^T[:, i*N+j] = relu(G1[:,i] + G2[:,j]); O^T = relu(Wl1@T^T + r_b)
        # is written per batch as [128, NN] feature-major; the host
        # transposes + drops diagonals while unsharding (a device-side
        # transpose would burn PE cycles and shatter the output DMA into
        # 512-byte descriptors).
        tpool = ctx.enter_context(tc.tile_pool(name="tT", bufs=2))
        oT_pool = ctx.enter_context(tc.tile_pool(name="oT", bufs=2))
        poT = ctx.enter_context(tc.tile_pool(name="poT", bufs=3, space="PSUM"))

        n_chunks = NN // 1024   # 9 double-bank chunks of 1024 pair-columns
        n_pieces = 3            # construct granularity: 32 i's = 3072 cols
        ip = N // n_pieces      # i's per piece
        pw = ip * N             # cols per piece
        for b in range(B_LOC):
            tT = tpool.tile([128, NN], FINDT, tag="tT")
            g1b = g1_sb[:, b * N:(b + 1) * N]
            g2b = g2_sb[:, b * N:(b + 1) * N]
            # T = relu(G1 (bcast over j) + G2 (bcast over i)) in a single
            # fused custom-DVE pass per piece
            g2rep = g2b.unsqueeze(1).broadcast_to([128, ip, N])
            for p in range(n_pieces):
                piece = tT[:, p * pw:(p + 1) * pw]
                out3d = piece.rearrange("q (i j) -> q i j", j=N)
                g1rep = (g1b[:, p * ip:(p + 1) * ip]
                         .unsqueeze(2).broadcast_to([128, ip, N]))
                nc.vector._custom_dve(_get_add_relu_op(), out=out3d,
                                      in0=g1rep, in1=g2rep)

            oT = oT_pool.tile([128, NN], F32, tag="oT")
            for c in range(n_chunks):
                oT_ps = poT.tile([128, 1024], F32, tag="oTps")
                for hh in range(2):
                    nc.tensor.matmul(
                        oT_ps[:, hh * 512:(hh + 1) * 512], (wl1T_sb[:]),
                        (tT[:, c * 1024 + hh * 512: c * 1024 + (hh + 1) * 512]),
                        start=True, stop=True)
                # relu + per-batch bias r while evicting PSUM->SBUF
                dst = oT[:, c * 1024:(c + 1) * 1024]
                nc.scalar.activation(dst, oT_ps[:], AF.Relu,
                                     bias=rT_sb[:, b:b + 1], scale=1.0)
                nc.sync.dma_start(
                    out[b][:, c * 1024:(c + 1) * 1024], dst)


_PROGRAM = None


def _build_program():
    global _PROGRAM
    if _PROGRAM is not None:
        return _PROGRAM
    nc = bacc.Bacc("TRN2", target_bir_lowering=False, debug=False)
    aps = (
        nc.dram_tensor("node", [NODE_DIM, 128 + NODES_LOC], PRODT,
                       kind="ExternalInput").ap(),
        nc.dram_tensor("wjT", [E, 2 * E], F32, kind="ExternalInput").ap(),
        nc.dram_tensor("wl1T", [E, E], FINDT, kind="ExternalInput").ap(),
        nc.dram_tensor("bias", [E, 2 + B_LOC], F32, kind="ExternalInput").ap(),
        nc.dram_tensor("outT", [B_LOC, E, NN], F32, kind="ExternalOutput").ap(),
    )
    with tile.TileContext(nc) as tc:
        _emit(tc, nc, aps)
    nc.compile()
    _PROGRAM = nc
    return nc


def _norm_weight(v, g):
    return (v * (g / np.linalg.norm(v, axis=1))[:, None]).astype(np.float32)


def _canonical_edges():
    i, j = np.meshgrid(np.arange(N), np.arange(N), indexing="ij")
    pair = (i * N + j)[i != j]                       # [N*(N-1)]
    edges = (np.arange(B)[:, None] * NN + pair[None, :]).reshape(-1)
    return edges.astype(np.int32), pair


def _reference_numpy(node_feats, cond_feats, edge_indexes, Wn, bn, Wj, bj,
                     Wq, bq, Wl, bl):
    """general fallback for non-canonical edge index tensors (host compute)"""
    h = np.maximum(node_feats.reshape(-1, NODE_DIM) @ Wn.T + bn, 0.0)
    e = edge_indexes.astype(np.int64)
    bi = e // NN
    rem = e % NN
    ii = rem // N
    jj = rem % N
    fusion = np.concatenate([h[bi * N + ii], h[bi * N + jj]], axis=1)
    joint = np.maximum(fusion @ Wj.T + bj, 0.0)
    q = np.maximum(cond_feats @ Wq.T + bq, 0.0)
    e_per = joint.shape[0] // B
    r = q @ Wl[:, E:].T + bl
    out = np.maximum(
        joint @ Wl[:, :E].T + np.repeat(r, e_per, axis=0), 0.0)
    return out.astype(np.float32)


def kernel(node_feats, cond_feats, edge_indexes,
           node_v, node_g, node_b,
           joint_v, joint_g, joint_b,
           q_v, q_g, q_b,
           lin_v, lin_g, lin_b):
    global LAST_RESULT, LAST_EXEC_NS
    node_feats = np.asarray(node_feats, dtype=np.float32)
    cond_feats = np.asarray(cond_feats, dtype=np.float32)
    edge_indexes = np.asarray(edge_indexes)

    Wn = _norm_weight(np.asarray(node_v, np.float32), np.asarray(node_g, np.float32))
    Wj = _norm_weight(np.asarray(joint_v, np.float32), np.asarray(joint_g, np.float32))
    Wq = _norm_weight(np.asarray(q_v, np.float32), np.asarray(q_g, np.float32))
    Wl = _norm_weight(np.asarray(lin_v, np.float32), np.asarray(lin_g, np.float32))
    bn = np.asarray(node_b, np.float32)
    bj = np.asarray(joint_b, np.float32)
    bq = np.asarray(q_b, np.float32)
    bl = np.asarray(lin_b, np.float32)

    canon, pair = _canonical_edges()
    if edge_indexes.shape != canon.shape or not np.array_equal(
            edge_indexes.astype(np.int64), canon.astype(np.int64)):
        print("kernel: non-canonical edge_indexes -> host fallback path")
        return _reference_numpy(node_feats, cond_feats, edge_indexes,
                                Wn, bn, Wj, bj, Wq, bq, Wl, bl)

    # host-side tiny precompute: per-batch bias r = q @ Wl2.T + bl  [B, E]
    q = np.maximum(cond_feats @ Wq.T + bq, 0.0)
    r = (q @ Wl[:, E:].T + bl).astype(np.float32)

    wnT = np.ascontiguousarray(Wn.T)                  # [2048, 128]
    wjT = np.concatenate([Wj[:, :E].T, Wj[:, E:].T], axis=1).copy()  # [128,256]
    wl1T = np.ascontiguousarray(Wl[:, :E].T)

    nc = _build_program()
    in_maps = []
    for c in range(N_CORES):
        # per k-chunk packed [wnT_k | nodeT_k], feature-major
        nodeT = node_feats[c * B_LOC:(c + 1) * B_LOC].reshape(
            NODES_LOC, NODE_DIM).T.reshape(K_CHUNKS, 128, NODES_LOC)
        comb = np.concatenate(
            [wnT.reshape(K_CHUNKS, 128, E), nodeT], axis=2)
        comb = np.ascontiguousarray(comb.reshape(NODE_DIM, 128 + NODES_LOC))
        rT = r[c * B_LOC:(c + 1) * B_LOC].T               # [128, 4]
        bias = np.concatenate(
            [bn.reshape(E, 1), bj.reshape(E, 1), rT], axis=1).copy()
        in_maps.append({
            "node": comb, "wjT": wjT, "wl1T": wl1T, "bias": bias,
        })

    do_trace = TRACE and _install_profile_hook()
    res = run_bass_kernel_spmd(nc, in_maps, core_ids=list(range(N_CORES)),
                               trace=do_trace)
    LAST_RESULT = res
    LAST_EXEC_NS = res.exec_time_ns

    # unshard: cores x [4, 128, 9216] feature-major -> row-major, drop
    # the diagonal pair slots, flatten
    full = np.concatenate([m["outT"] for m in res.results], axis=0)
    out = full.transpose(0, 2, 1)[:, pair, :].reshape(B * EDGES_PER_B, E)
    return np.ascontiguousarray(out)
